# revision 1
# baseline (speedup 1.0000x reference)
"""Causal self-attention (B=4, T=2048, C=1024, H=16) on 8 trn2 NeuronCores.

Sharding: core = 2*b + g  (b = batch 0..3, g = head-group 0..1).
Each core handles 1 batch and 8 heads (global heads 8g..8g+7) and returns a
partial projection output [T, C]; the host sums the two head-group partials
per batch.

Per-core dataflow (all feature-major / transposed layouts so every matmul
contraction sits on the partition axis; no on-device transposes needed):
  P1  qkT = wqkT.T @ xT            -> Q,K per head as [64(d), T] (f32)
  P2  vT  = xT.T @ wvT             -> V per head as [T, 64(d)] directly
  P3  per (head-pair, q-tile): S_T = K.T Q (row-packed pairs), E = exp(S/8)
      (causal: skip blocks above diagonal, triangle-mask diagonal blocks),
      [y; sums] accumulated via [VT | ones] lhsT; normalize with
      reciprocal + gpsimd partition_broadcast.
  P4  out = y.T @ wpT (accumulate over heads), DMA partial result out.
"""

import numpy as np
import ml_dtypes

B, T, C = 4, 2048, 1024
H_LOC = 8          # heads per core
HD = 64            # head dim
N_CORES = 8
QT = 512           # q tile (free dim of S_T)
KT = 128           # k tile (partition dim of S_T)
NQT = T // QT      # 4
NKT = T // KT      # 16
NCT = C // 128     # 8 contraction tiles for qkv

bf16 = ml_dtypes.bfloat16

_CACHE = {}


def _build():
    import concourse.bacc as bacc
    import concourse.tile as tile
    import concourse.mybir as mybir
    from contextlib import ExitStack

    f32 = mybir.dt.float32
    f32r = mybir.dt.float32r
    b16 = mybir.dt.bfloat16
    EXP = mybir.ActivationFunctionType.Exp

    nc = bacc.Bacc("TRN2", target_bir_lowering=False, debug=False)

    xT_d = nc.dram_tensor("xT", [C, T], b16, kind="ExternalInput").ap()
    wqk_d = nc.dram_tensor("wqk", [C, 2 * H_LOC * HD], b16, kind="ExternalInput").ap()
    wv_d = nc.dram_tensor("wv", [C, H_LOC * HD], b16, kind="ExternalInput").ap()
    wp_d = nc.dram_tensor("wp", [H_LOC * HD, C], b16, kind="ExternalInput").ap()
    tri_d = nc.dram_tensor("tri", [128, 128], b16, kind="ExternalInput").ap()
    out_d = nc.dram_tensor("out", [T, C], f32, kind="ExternalOutput").ap()

    with tile.TileContext(nc) as tc:
        with ExitStack() as ctx:
            # ---- persistent SBUF ----
            pers = ctx.enter_context(tc.tile_pool(name="pers", bufs=1))
            qk_sb = pers.tile([128, 8, T], f32r)         # jt 0..3 Q pairs, 4..7 K pairs
            vt_sb = pers.tile([128, NKT, H_LOC, HD + 2], b16)  # +2: ones col at 64, pad at 65 (4B align)
            y_un = [pers.tile([HD + 1, T], b16, name=f"yu{h}") for h in range(H_LOC)]
            tri_sb = pers.tile([128, 128], b16)

            nc.gpsimd.memset(vt_sb[:, :, :, HD], 1.0)    # ones col for row sums
            nc.sync.dma_start(tri_sb, tri_d)

            # ---- transient inputs ----
            trans = tc.alloc_tile_pool(name="trans", bufs=1)
            xT_sb = trans.tile([128, NCT, T], b16)
            wqk_sb = trans.tile([128, NCT, 2 * H_LOC * HD], b16)
            wv_sb = trans.tile([128, NCT, H_LOC * HD], b16)
            xT_r = xT_d.rearrange("(a p) t -> p a t", p=128)
            wqk_r = wqk_d.rearrange("(a p) j -> p a j", p=128)
            wv_r = wv_d.rearrange("(a p) j -> p a j", p=128)
            for a in range(NCT):
                nc.sync.dma_start(wqk_sb[:, a, :], wqk_r[:, a, :])
                nc.sync.dma_start(xT_sb[:, a, :], xT_r[:, a, :])
                nc.sync.dma_start(wv_sb[:, a, :], wv_r[:, a, :])

            # ---- P1: Q,K projections (feature-major output) ----
            with tc.tile_pool(name="ps12", bufs=2, space="PSUM") as ps12:
                for jt in range(8):
                    for tt in range(NQT):
                        ps = ps12.tile([128, QT], f32, name="p1", tag="p1")
                        for a in range(NCT):
                            nc.tensor.matmul(
                                ps,
                                lhsT=wqk_sb[:, a, jt * 128:(jt + 1) * 128],
                                rhs=xT_sb[:, a, tt * QT:(tt + 1) * QT],
                                start=(a == 0), stop=(a == NCT - 1))
                        nc.vector.tensor_copy(qk_sb[:, jt, tt * QT:(tt + 1) * QT], ps)
                # ---- P2: V, directly transposed ([T, 64] per head) ----
                for kt in range(NKT):
                    ps = ps12.tile([128, H_LOC * HD], f32, name="p2", tag="p1")
                    for a in range(NCT):
                        nc.tensor.matmul(
                            ps,
                            lhsT=xT_sb[:, a, kt * 128:(kt + 1) * 128],
                            rhs=wv_sb[:, a, :],
                            start=(a == 0), stop=(a == NCT - 1))
                    nc.vector.tensor_copy(
                        vt_sb[:, kt, :, 0:HD],
                        ps.rearrange("p (h d) -> p h d", d=HD))

            trans.release()

            # ---- P3: attention ----
            wpool = ctx.enter_context(tc.tile_pool(name="wpool", bufs=1))
            wp_sb = [wpool.tile([HD, C], b16, name=f"wp{h}") for h in range(H_LOC)]
            for h in range(H_LOC):
                nc.sync.dma_start(wp_sb[h], wp_d[h * HD:(h + 1) * HD, :])
            epool = ctx.enter_context(tc.tile_pool(name="epool", bufs=3))
            npool = ctx.enter_context(tc.tile_pool(name="npool", bufs=2))
            psS = tc.alloc_tile_pool(name="psS", bufs=2, space="PSUM")
            psY = tc.alloc_tile_pool(name="psY", bufs=1, space="PSUM")
            psO = tc.alloc_tile_pool(name="psO", bufs=2, space="PSUM")

            for p in range(4):              # head pairs (2p, 2p+1)
                qA = qk_sb[0:64, p, :]
                qB = qk_sb[64:128, p, :]
                kA = qk_sb[0:64, 4 + p, :]
                kB = qk_sb[64:128, 4 + p, :]
                for qt in range(NQT):
                    yA = psY.tile([128, QT], f32, name="yA", tag="yA")
                    yB = psY.tile([128, QT], f32, name="yB", tag="yB")
                    kt_hi = 4 * qt + 3
                    for kt in range(kt_hi + 1):
                        r = kt - 4 * qt
                        c0 = 128 * r if r > 0 else 0
                        sA = psS.tile([128, QT], f32, name="sA", tag="sA")
                        sB = psS.tile([128, QT], f32, name="sB", tag="sB")
                        nc.tensor.matmul(
                            sA[:, c0:], lhsT=kA[:, kt * 128:(kt + 1) * 128],
                            rhs=qA[:, qt * QT + c0:(qt + 1) * QT],
                            start=True, stop=True)
                        nc.tensor.matmul(
                            sB[:, c0:], lhsT=kB[:, kt * 128:(kt + 1) * 128],
                            rhs=qB[:, qt * QT + c0:(qt + 1) * QT],
                            start=True, stop=True)
                        eA = epool.tile([128, QT], b16, name="eA", tag="eA")
                        eB = epool.tile([128, QT], b16, name="eB", tag="eB")
                        nc.scalar.activation(eA[:, c0:], sA[:, c0:], EXP, scale=0.125)
                        nc.scalar.activation(eB[:, c0:], sB[:, c0:], EXP, scale=0.125)
                        if r >= 0:
                            nc.vector.tensor_mul(
                                eA[:, c0:c0 + 128], eA[:, c0:c0 + 128], tri_sb)
                            nc.vector.tensor_mul(
                                eB[:, c0:c0 + 128], eB[:, c0:c0 + 128], tri_sb)
                        nc.tensor.matmul(
                            yA[0:HD + 1, c0:], lhsT=vt_sb[:, kt, 2 * p, 0:HD + 1],
                            rhs=eA[:, c0:], start=(kt == 0), stop=(kt == kt_hi))
                        nc.tensor.matmul(
                            yB[0:HD + 1, c0:], lhsT=vt_sb[:, kt, 2 * p + 1, 0:HD + 1],
                            rhs=eB[:, c0:], start=(kt == 0), stop=(kt == kt_hi))
                    # stage unnormalized y + rowsums (row 64) to SBUF
                    for h, y in ((2 * p, yA), (2 * p + 1, yB)):
                        nc.vector.tensor_copy(
                            y_un[h][:, qt * QT:(qt + 1) * QT], y[0:HD + 1, :])

                # normalize this pair: recip of rowsums (on partitions 0-1)
                sp_sums = npool.tile([2, T], f32, name="sp_sums", tag="sp_sums")
                sp_rec = npool.tile([2, T], f32, name="sp_rec", tag="sp_rec")
                sp_scr = npool.tile([2, T], f32, name="sp_scr", tag="sp_scr")
                for i, h in enumerate((2 * p, 2 * p + 1)):
                    nc.gpsimd.dma_start(sp_sums[i:i + 1, :], y_un[h][HD:HD + 1, :])
                nc.vector.reciprocal_approx_accurate(
                    out=sp_rec, in_=sp_sums, scratch=sp_scr)
                for i, h in enumerate((2 * p, 2 * p + 1)):
                    rc0 = npool.tile([1, T], b16, name="rc0", tag="rc0")
                    nc.gpsimd.dma_start(rc0, sp_rec[i:i + 1, :])
                    bs = npool.tile([HD, T], b16, name="bs", tag="bs")
                    nc.gpsimd.partition_broadcast(bs, rc0)
                    nc.vector.tensor_mul(y_un[h][0:HD, :], y_un[h][0:HD, :], bs)

            # ---- P4: output projection (partial over local heads) ----
            spool = ctx.enter_context(tc.tile_pool(name="spool", bufs=4))
            for tt in range(T // 128):
                for ot in range(C // 512):
                    ps = psO.tile([128, 512], f32, name="po", tag="po")
                    for h in range(H_LOC):
                        nc.tensor.matmul(
                            ps, lhsT=y_un[h][0:HD, tt * 128:(tt + 1) * 128],
                            rhs=wp_sb[h][:, ot * 512:(ot + 1) * 512],
                            start=(h == 0), stop=(h == H_LOC - 1))
                    st = spool.tile([128, 512], f32, name="st", tag="st")
                    nc.vector.tensor_copy(st, ps)
                    nc.sync.dma_start(
                        out_d[tt * 128:(tt + 1) * 128, ot * 512:(ot + 1) * 512], st)
            psO.release()
            psY.release()
            psS.release()

    nc.compile()
    return nc


def _prep_inputs(x, w_attn, w_proj):
    # tri[kl, ql] = 1 if ql >= kl (keep), else 0 (causal-masked)
    tri = np.ascontiguousarray(np.triu(np.ones((128, 128), np.float32))).astype(bf16)
    in_maps = []
    for core in range(N_CORES):
        b, g = core // 2, core % 2
        heads = [8 * g + i for i in range(H_LOC)]
        q_rows = np.concatenate([w_attn[HD * h:HD * h + HD] for h in heads])
        k_rows = np.concatenate([w_attn[C + HD * h:C + HD * h + HD] for h in heads])
        v_rows = np.concatenate([w_attn[2 * C + HD * h:2 * C + HD * h + HD] for h in heads])
        wqk = np.ascontiguousarray(np.concatenate([q_rows, k_rows]).T).astype(bf16)
        wv = np.ascontiguousarray(v_rows.T).astype(bf16)
        wp = np.ascontiguousarray(
            np.concatenate([w_proj[:, HD * h:HD * h + HD] for h in heads], axis=1).T
        ).astype(bf16)
        xT = np.ascontiguousarray(x[b].T).astype(bf16)
        in_maps.append({"xT": xT, "wqk": wqk, "wv": wv, "wp": wp, "tri": tri})
    return in_maps


def kernel(x, w_attn, w_proj):
    from concourse.bass_utils import run_bass_kernel_spmd

    x = np.asarray(x, dtype=np.float32)
    w_attn = np.asarray(w_attn, dtype=np.float32)
    w_proj = np.asarray(w_proj, dtype=np.float32)

    if "nc" not in _CACHE:
        _CACHE["nc"] = _build()
    nc = _CACHE["nc"]

    in_maps = _prep_inputs(x, w_attn, w_proj)
    res = run_bass_kernel_spmd(nc, in_maps, core_ids=list(range(N_CORES)))
    outs = [res.results[c]["out"] for c in range(N_CORES)]
    y = np.stack([outs[2 * b] + outs[2 * b + 1] for b in range(B)])
    return y.astype(np.float32)



# revision 2
# speedup vs baseline: 1.3381x; 1.3381x over previous
"""Causal self-attention (B=4, T=2048, C=1024, H=16) on 8 trn2 NeuronCores.

Sharding: core = 2*b + g  (b = batch 0..3, g = head-group 0..1).
Each core handles 1 batch and 8 heads and returns a partial projection
output [T, C]; the host sums the two head-group partials per batch.

v2 dataflow (flash-style q-superstep ordering, transposed-Y attention):
  For each 512-token q-superstep i:
    P1  project Q,K (feature-major [128j, t]) and V (token-major [t, j])
        for this t-range (interleaved as PE filler into step i-1's attention)
    P3  per (head, kt<=4i+3): S_T = K.T Q  [128k, 512q], E = exp(S/8) (Act),
        tri-mask diagonal blocks (DVE), then per 128-q subtile:
        y[q, 64d+1] += E_blk.T @ [V|1]  (cost 65/row-block vs 128 feature-major)
        normalize: one reciprocal [128,4] per (head,step) + DVE scalar-mul
    P5  transpose y [t, j] -> [j, t] via PE (cheap), P4 out = yT.T @ wp with
        128-deep contraction chunks (head pairs packed), stream out per t-tile.
"""

import numpy as np
import ml_dtypes

B, T, C = 4, 2048, 1024
H_LOC = 8          # heads per core
HD = 64            # head dim
N_CORES = 8
QT = 512           # q super-step width
NST = T // QT      # 4
NCT = C // 128     # 8 contraction chunks for qkv

bf16 = ml_dtypes.bfloat16

_CACHE = {}


def _build():
    import concourse.bacc as bacc
    import concourse.tile as tile
    import concourse.mybir as mybir
    from contextlib import ExitStack

    f32 = mybir.dt.float32
    b16 = mybir.dt.bfloat16
    EXP = mybir.ActivationFunctionType.Exp

    nc = bacc.Bacc("TRN2", target_bir_lowering=False, debug=False)

    xT_d = nc.dram_tensor("xT", [C, T], b16, kind="ExternalInput").ap()
    wqk_d = nc.dram_tensor("wqk", [C, 2 * H_LOC * HD], b16, kind="ExternalInput").ap()
    wv_d = nc.dram_tensor("wv", [C, H_LOC * HD], b16, kind="ExternalInput").ap()
    wp_d = nc.dram_tensor("wp", [H_LOC * HD, C], b16, kind="ExternalInput").ap()
    tri_d = nc.dram_tensor("tri", [128, 128], b16, kind="ExternalInput").ap()
    ident_d = nc.dram_tensor("ident", [128, 128], b16, kind="ExternalInput").ap()
    out_d = nc.dram_tensor("out", [T, C], f32, kind="ExternalOutput").ap()

    with tile.TileContext(nc) as tc:
        with ExitStack() as ctx:
            pers = ctx.enter_context(tc.tile_pool(name="pers", bufs=1))
            xT_sb = pers.tile([128, NCT, T], b16)
            wqk_sb = pers.tile([128, NCT, 2 * H_LOC * HD], b16)
            wv_sb = pers.tile([128, NCT, H_LOC * HD], b16)
            wp_sb = pers.tile([128, 4, C], b16)
            tri_sb = pers.tile([128, 128], b16)
            id_sb = pers.tile([128, 128], b16)
            qk_sb = pers.tile([128, 8, T], b16)   # jt 0..3 Q pairs, 4..7 K pairs
            vt_sb = pers.tile([128, T // 128, H_LOC, HD + 2], b16)  # ones col @64

            nc.gpsimd.memset(vt_sb[:, :, :, HD], 1.0)

            xT_r = xT_d.rearrange("(a p) t -> p a t", p=128)
            wqk_r = wqk_d.rearrange("(a p) j -> p a j", p=128)
            wv_r = wv_d.rearrange("(a p) j -> p a j", p=128)
            wp_r = wp_d.rearrange("(c p) j -> p c j", p=128)
            nc.sync.dma_start(tri_sb, tri_d)
            nc.sync.dma_start(id_sb, ident_d)
            for a in range(NCT):
                nc.sync.dma_start(wqk_sb[:, a, :], wqk_r[:, a, :])
            for a in range(NCT):
                nc.sync.dma_start(xT_sb[:, a, 0:QT], xT_r[:, a, 0:QT])
                nc.sync.dma_start(wv_sb[:, a, :], wv_r[:, a, :])
            for i in range(1, NST):
                for a in range(NCT):
                    nc.sync.dma_start(
                        xT_sb[:, a, i * QT:(i + 1) * QT], xT_r[:, a, i * QT:(i + 1) * QT])
            for c4 in range(4):
                nc.sync.dma_start(wp_sb[:, c4, :], wp_r[:, c4, :])

            epool = ctx.enter_context(tc.tile_pool(name="epool", bufs=4))
            ysb_pool = ctx.enter_context(tc.tile_pool(name="ysb", bufs=8))
            ytp_pool = ctx.enter_context(tc.tile_pool(name="ytp", bufs=2))
            rpool = ctx.enter_context(tc.tile_pool(name="rpool", bufs=4))
            spool = ctx.enter_context(tc.tile_pool(name="spool", bufs=4))
            ps_big = ctx.enter_context(tc.tile_pool(name="psbig", bufs=3, space="PSUM"))
            ps_s = ctx.enter_context(tc.tile_pool(name="pss", bufs=2, space="PSUM"))
            ps_y = ctx.enter_context(tc.tile_pool(name="psy", bufs=2, space="PSUM"))
            ps_t = ctx.enter_context(tc.tile_pool(name="pst", bufs=1, space="PSUM"))

            y_tiles = [None] * (T // 128)

            def proj_step(i):
                groups = []
                t0 = i * QT

                def qk_group(jt, t0=t0):
                    ps = ps_big.tile([128, QT], f32, name="pqk", tag="big")
                    for a in range(NCT):
                        nc.tensor.matmul(
                            ps, lhsT=wqk_sb[:, a, jt * 128:(jt + 1) * 128],
                            rhs=xT_sb[:, a, t0:t0 + QT],
                            start=(a == 0), stop=(a == NCT - 1))
                    nc.vector.tensor_copy(qk_sb[:, jt, t0:t0 + QT], ps)

                def v_group(tt, i=i):
                    tg = 4 * i + tt
                    ps = ps_big.tile([128, H_LOC * HD], f32, name="pv", tag="big")
                    for a in range(NCT):
                        nc.tensor.matmul(
                            ps, lhsT=xT_sb[:, a, tg * 128:(tg + 1) * 128],
                            rhs=wv_sb[:, a, :],
                            start=(a == 0), stop=(a == NCT - 1))
                    nc.vector.tensor_copy(
                        vt_sb[:, tg, :, 0:HD], ps.rearrange("p (h d) -> p h d", d=HD))

                for jt in range(8):
                    groups.append(lambda jt=jt: qk_group(jt))
                for tt in range(4):
                    groups.append(lambda tt=tt: v_group(tt))
                return groups

            def tp_p4_step(i):
                groups = []

                def g(tt, i=i):
                    tg = 4 * i + tt
                    yflat = y_tiles[tg].rearrange("p h d -> p (h d)")
                    tp = ps_t.tile([128, 4, 128], b16, name="tp", tag="tp")
                    for jc in range(4):
                        nc.tensor.transpose(
                            tp[:, jc, :], yflat[:, jc * 128:(jc + 1) * 128], id_sb)
                    yt = ytp_pool.tile([128, 4, 128], b16, name="yt", tag="yt")
                    nc.vector.tensor_copy(yt, tp)
                    for ot in range(2):
                        ps = ps_big.tile([128, 512], f32, name="po", tag="big")
                        for jc in range(4):
                            nc.tensor.matmul(
                                ps, lhsT=yt[:, jc, :],
                                rhs=wp_sb[:, jc, ot * 512:(ot + 1) * 512],
                                start=(jc == 0), stop=(jc == 3))
                        st = spool.tile([128, 512], f32, name="st", tag="st")
                        nc.vector.tensor_copy(st, ps)
                        nc.sync.dma_start(
                            out_d[tg * 128:(tg + 1) * 128, ot * 512:(ot + 1) * 512], st)

                for tt in range(4):
                    groups.append(lambda tt=tt: g(tt))
                return groups

            def attn_head(i, p, hh):
                h = 2 * p + hh
                kt_hi = 4 * i + 3
                q_ap = qk_sb[hh * 64:(hh + 1) * 64, p, :]
                k_ap = qk_sb[hh * 64:(hh + 1) * 64, 4 + p, :]
                y_ps = ps_y.tile([128, 4, 66], f32, name="yps", tag="yps")
                for kt in range(kt_hi + 1):
                    r = kt - 4 * i
                    c0 = 128 * r if r > 0 else 0
                    s_ps = ps_s.tile([128, QT], f32, name="sps", tag="sps")
                    nc.tensor.matmul(
                        s_ps[:, c0:], lhsT=k_ap[:, kt * 128:(kt + 1) * 128],
                        rhs=q_ap[:, i * QT + c0:(i + 1) * QT],
                        start=True, stop=True)
                    e = epool.tile([128, QT], b16, name="e", tag="e")
                    nc.scalar.activation(e[:, c0:], s_ps[:, c0:], EXP, scale=0.125)
                    if r >= 0:
                        nc.vector.tensor_mul(e[:, c0:c0 + 128], e[:, c0:c0 + 128], tri_sb)
                    for sub in range(max(0, r), 4):
                        nc.tensor.matmul(
                            y_ps[:, sub, 0:HD + 1],
                            lhsT=e[:, sub * 128:(sub + 1) * 128],
                            rhs=vt_sb[:, kt, h, 0:HD + 1],
                            start=(kt == 0 and sub == max(0, r)),
                            stop=(kt == kt_hi and sub == 3))
                rc = rpool.tile([128, 4], f32, name="rc", tag="rc")
                nc.vector.reciprocal(rc, y_ps[:, :, HD])
                for sub in range(4):
                    tg = 4 * i + sub
                    nc.vector.tensor_scalar_mul(
                        y_tiles[tg][:, h, :], y_ps[:, sub, 0:HD], rc[:, sub:sub + 1])

            head_order = [(p, hh) for p in range(4) for hh in range(2)]
            for g in proj_step(0):
                g()
            pending = []
            for i in range(NST):
                for tt in range(4):
                    y_tiles[4 * i + tt] = ysb_pool.tile(
                        [128, H_LOC, HD], b16, name="ysb", tag="ysb")
                fill = (proj_step(i + 1) if i + 1 < NST else []) + pending
                pending = []
                nf = len(fill)
                for idx, (p, hh) in enumerate(head_order):
                    attn_head(i, p, hh)
                    for g in fill[idx * nf // 8:(idx + 1) * nf // 8]:
                        g()
                pending = tp_p4_step(i)
            for g in pending:
                g()

    nc.compile()
    return nc


def _prep_inputs(x, w_attn, w_proj):
    # tri[kl, ql] = 1 if ql >= kl (keep), else 0 (causal-masked)
    tri = np.ascontiguousarray(np.triu(np.ones((128, 128), np.float32))).astype(bf16)
    ident = np.ascontiguousarray(np.eye(128, dtype=np.float32)).astype(bf16)
    in_maps = []
    for core in range(N_CORES):
        b, g = core // 2, core % 2
        heads = [8 * g + i for i in range(H_LOC)]
        q_rows = np.concatenate([w_attn[HD * h:HD * h + HD] for h in heads])
        k_rows = np.concatenate([w_attn[C + HD * h:C + HD * h + HD] for h in heads])
        v_rows = np.concatenate([w_attn[2 * C + HD * h:2 * C + HD * h + HD] for h in heads])
        wqk = np.ascontiguousarray(np.concatenate([q_rows, k_rows]).T).astype(bf16)
        wv = np.ascontiguousarray(v_rows.T).astype(bf16)
        wp = np.ascontiguousarray(
            np.concatenate([w_proj[:, HD * h:HD * h + HD] for h in heads], axis=1).T
        ).astype(bf16)
        xT = np.ascontiguousarray(x[b].T).astype(bf16)
        in_maps.append(
            {"xT": xT, "wqk": wqk, "wv": wv, "wp": wp, "tri": tri, "ident": ident})
    return in_maps


def kernel(x, w_attn, w_proj):
    from concourse.bass_utils import run_bass_kernel_spmd

    x = np.asarray(x, dtype=np.float32)
    w_attn = np.asarray(w_attn, dtype=np.float32)
    w_proj = np.asarray(w_proj, dtype=np.float32)

    if "nc" not in _CACHE:
        _CACHE["nc"] = _build()
    nc = _CACHE["nc"]

    in_maps = _prep_inputs(x, w_attn, w_proj)
    res = run_bass_kernel_spmd(nc, in_maps, core_ids=list(range(N_CORES)))
    outs = [res.results[c]["out"] for c in range(N_CORES)]
    y = np.stack([outs[2 * b] + outs[2 * b + 1] for b in range(B)])
    return y.astype(np.float32)


# revision 9
# speedup vs baseline: 1.5262x; 1.1406x over previous
"""Causal self-attention (B=4, T=2048, C=1024, H=16) on 8 trn2 NeuronCores.

Sharding: core = 2*b + g  (b = batch 0..3, g = head-group 0..1).
Each core handles 1 batch and 8 heads and returns a partial projection
output [T, C]; the host sums the two head-group partials per batch.

v3: flash-style q-superstep ordering + transposed-Y attention + fp8
DoubleRow 3-term-compensated projections (QK, V, P4) + deficit-scheduled
PE filler interleave so projection/P4 matmuls hide the Act-engine-bound
exp phase.

Per-superstep i (512 q):
  P1  project Q,K (feature-major [128j, t]) and V (token-major [t, j]) for
      this t-range via fp8 DR matmuls:  x_hi@W_hi + x_lo@W_hi + x_hi@W_lo,
      each term 4 DoubleRow matmuls (256-deep contraction, 0.5 cyc/row).
  P3  per (head, kt<=4i+3): S_T = K.T Q [128k, 512q] (bf16), E = exp(S/8)
      on Act, tri-mask diag blocks on DVE; per 128-q subtile accumulate
      y[q, 64d | rowsum] += E_blk.T @ [V|1] (transposed-Y: 65 rows/block).
      S runs one kt ahead of Y; fillers (future proj / deferred P4) are
      drained whenever the Act-minus-PE deficit estimate goes positive.
      Normalize: one DVE reciprocal [128,4] per (head, step) + scalar-mul.
  P5  (deferred, as filler) transpose y [t,j]->[j,t] on PE, split yT into
      fp8 hi/lo on DVE, P4 out = yT.T @ wp via fp8 DR matmuls, stream out.
"""

import numpy as np
import ml_dtypes

B, T, C = 4, 2048, 1024
H_LOC = 8          # heads per core
HD = 64            # head dim
N_CORES = 8
QT = 512           # q super-step width
NST = T // QT      # 4
NCT = C // 128     # 8 contraction chunks for qkv

bf16 = ml_dtypes.bfloat16
f8 = ml_dtypes.float8_e4m3

# fp8 hi/lo splits are pre-scaled so the lo residuals stay in e4m3's normal
# range (unscaled weight residuals ~1e-3 fall below the 2^-9 min subnormal
# and flush to zero, killing the compensation).  Products carry SX*SW=512;
# PSUM evacuations divide it back out.
SX = 8.0           # x pre-scale
SW = 64.0          # weight pre-scale
SY = 8.0           # on-device y pre-scale for the P4 hi/lo split

PE_NS = 0.417      # ns per output row at full pstate
ACT_NS = 0.833
ACT_OVH = 185.0

_CACHE = {}


def _build():
    import concourse.bacc as bacc
    import concourse.tile as tile
    import concourse.mybir as mybir
    from contextlib import ExitStack

    f32 = mybir.dt.float32
    b16 = mybir.dt.bfloat16
    fp8 = mybir.dt.float8e4
    EXP = mybir.ActivationFunctionType.Exp
    DR = mybir.MatmulPerfMode.DoubleRow

    nc = bacc.Bacc("TRN2", target_bir_lowering=False, debug=False)

    xh_d = nc.dram_tensor("xh", [C, T], fp8, kind="ExternalInput").ap()
    xl_d = nc.dram_tensor("xl", [C, T], fp8, kind="ExternalInput").ap()
    wqkh_d = nc.dram_tensor("wqkh", [C, 1024], fp8, kind="ExternalInput").ap()
    wqkl_d = nc.dram_tensor("wqkl", [C, 1024], fp8, kind="ExternalInput").ap()
    wvh_d = nc.dram_tensor("wvh", [C, 512], fp8, kind="ExternalInput").ap()
    wvl_d = nc.dram_tensor("wvl", [C, 512], fp8, kind="ExternalInput").ap()
    wph_d = nc.dram_tensor("wph", [512, C], fp8, kind="ExternalInput").ap()
    wpl_d = nc.dram_tensor("wpl", [512, C], fp8, kind="ExternalInput").ap()
    tri_d = nc.dram_tensor("tri", [128, 128], b16, kind="ExternalInput").ap()
    ident_d = nc.dram_tensor("ident", [128, 128], b16, kind="ExternalInput").ap()
    out_d = nc.dram_tensor("out", [T, C], f32, kind="ExternalOutput").ap()

    with tile.TileContext(nc) as tc:
        with ExitStack() as ctx:
            pers = ctx.enter_context(tc.tile_pool(name="pers", bufs=1))
            xh_sb = pers.tile([128, NCT, T], fp8)
            xl_sb = pers.tile([128, NCT, T], fp8)
            wqkh_sb = pers.tile([128, NCT, 1024], fp8)
            wqkl_sb = pers.tile([128, NCT, 1024], fp8)
            wvh_sb = pers.tile([128, NCT, 512], fp8)
            wvl_sb = pers.tile([128, NCT, 512], fp8)
            wph_sb = pers.tile([128, 4, C], fp8)
            wpl_sb = pers.tile([128, 4, C], fp8)
            tri_sb = pers.tile([128, 128], b16)
            id_sb = pers.tile([128, 128], b16)
            qk_sb = pers.tile([128, 8, T], b16)   # jt 0..3 Q pairs, 4..7 K pairs
            vt_sb = pers.tile([128, T // 128, H_LOC, HD + 2], b16)  # ones col @64
            y_tiles = [pers.tile([128, H_LOC, HD], b16, name=f"y{t}")
                       for t in range(T // 128)]

            nc.gpsimd.memset(vt_sb[:, :, :, HD], 1.0)

            xh_r = xh_d.rearrange("(a p) t -> p a t", p=128)
            xl_r = xl_d.rearrange("(a p) t -> p a t", p=128)
            wqkh_r = wqkh_d.rearrange("(a p) j -> p a j", p=128)
            wqkl_r = wqkl_d.rearrange("(a p) j -> p a j", p=128)
            wvh_r = wvh_d.rearrange("(a p) j -> p a j", p=128)
            wvl_r = wvl_d.rearrange("(a p) j -> p a j", p=128)
            wph_r = wph_d.rearrange("(c p) j -> p c j", p=128)
            wpl_r = wpl_d.rearrange("(c p) j -> p c j", p=128)

            nc.sync.dma_start(tri_sb, tri_d)
            nc.sync.dma_start(id_sb, ident_d)
            # step-0 x first, then wqk in pair-first column order, wv, rest
            nc.sync.dma_start(xh_sb[:, :, 0:QT], xh_r[:, :, 0:QT])
            nc.sync.dma_start(xl_sb[:, :, 0:QT], xl_r[:, :, 0:QT])
            for jt in (0, 4, 1, 5, 2, 6, 3, 7):
                c = slice(jt * 128, (jt + 1) * 128)
                nc.sync.dma_start(wqkh_sb[:, :, c], wqkh_r[:, :, c])
                nc.sync.dma_start(wqkl_sb[:, :, c], wqkl_r[:, :, c])
                if jt == 4:
                    nc.sync.dma_start(wvh_sb, wvh_r)
                    nc.sync.dma_start(wvl_sb, wvl_r)
            for i in range(1, NST):
                s = slice(i * QT, (i + 1) * QT)
                nc.sync.dma_start(xh_sb[:, :, s], xh_r[:, :, s])
                nc.sync.dma_start(xl_sb[:, :, s], xl_r[:, :, s])
            nc.sync.dma_start(wph_sb, wph_r)
            nc.sync.dma_start(wpl_sb, wpl_r)

            epool = ctx.enter_context(tc.tile_pool(name="epool", bufs=4))
            ytp_pool = ctx.enter_context(tc.tile_pool(name="ytp", bufs=2))
            rpool = ctx.enter_context(tc.tile_pool(name="rpool", bufs=4))
            spool = ctx.enter_context(tc.tile_pool(name="spool", bufs=4))
            ps_big = ctx.enter_context(tc.tile_pool(name="psbig", bufs=3, space="PSUM"))
            ps_s = ctx.enter_context(tc.tile_pool(name="pss", bufs=2, space="PSUM"))
            ps_y = ctx.enter_context(tc.tile_pool(name="psy", bufs=2, space="PSUM"))
            ps_t = ctx.enter_context(tc.tile_pool(name="pst", bufs=1, space="PSUM"))

            # ---- deficit-based filler scheduler ----
            sched = {"d": 0.0}
            fq = []          # list of dicts: key, fn, pe

            def dr3(ps, lhs_h, lhs_l, rhs_h, rhs_l, n):
                """3-term fp8 DoubleRow GEMM over 1024-deep contraction.
                lhs*/rhs* are [128, NCT, n]-style slices; accumulate in ps."""
                terms = ((lhs_h, rhs_h), (lhs_l, rhs_h), (lhs_h, rhs_l))
                for ti, (lt, rt) in enumerate(terms):
                    for a4 in range(4):
                        sl = slice(2 * a4, 2 * a4 + 2)
                        nc.tensor.matmul(
                            ps, lhsT=lt[:, sl, :], rhs=rt[:, sl, :],
                            start=(ti == 0 and a4 == 0),
                            stop=(ti == 2 and a4 == 3),
                            perf_mode=DR)

            def qk_group(jt, i):
                t0 = i * QT
                c = slice(jt * 128, (jt + 1) * 128)
                s = slice(t0, t0 + QT)
                ps = ps_big.tile([128, QT], f32, name="pqk", tag="big")
                dr3(ps, wqkh_sb[:, :, c], wqkl_sb[:, :, c],
                    xh_sb[:, :, s], xl_sb[:, :, s], QT)
                nc.vector.tensor_scalar_mul(qk_sb[:, jt, s], ps, 1.0 / (SX * SW))
                sched["d"] -= 3072 * PE_NS

            def v_group(tt, i):
                tg = 4 * i + tt
                s = slice(tg * 128, (tg + 1) * 128)
                ps = ps_big.tile([128, H_LOC * HD], f32, name="pv", tag="big")
                dr3(ps, xh_sb[:, :, s], xl_sb[:, :, s], wvh_sb, wvl_sb, 512)
                nc.vector.tensor_scalar_mul(
                    vt_sb[:, tg, :, 0:HD], ps.rearrange("p (h d) -> p h d", d=HD),
                    1.0 / (SX * SW))
                sched["d"] -= 3072 * PE_NS

            def tp_p4(tg):
                yflat = y_tiles[tg].rearrange("p h d -> p (h d)")
                tp = ps_t.tile([128, 4, 128], b16, name="tp", tag="tp")
                for jc in range(4):
                    nc.tensor.transpose(
                        tp[:, jc, :], yflat[:, jc * 128:(jc + 1) * 128], id_sb)
                yth = ytp_pool.tile([128, 4, 128], fp8, name="yth", tag="yth")
                ytl = ytp_pool.tile([128, 4, 128], fp8, name="ytl", tag="ytl")
                nc.vector.tensor_scalar_mul(yth, tp, SY)
                nc.vector.scalar_tensor_tensor(
                    ytl, tp, SY, yth, mybir.AluOpType.mult, mybir.AluOpType.subtract)
                for ot in range(2):
                    so = slice(ot * 512, (ot + 1) * 512)
                    ps = ps_big.tile([128, 512], f32, name="po", tag="big")
                    terms = ((yth, wph_sb), (ytl, wph_sb), (yth, wpl_sb))
                    for ti, (lt, rt) in enumerate(terms):
                        for c4 in range(2):
                            sl = slice(2 * c4, 2 * c4 + 2)
                            nc.tensor.matmul(
                                ps, lhsT=lt[:, sl, :], rhs=rt[:, sl, so],
                                start=(ti == 0 and c4 == 0),
                                stop=(ti == 2 and c4 == 1),
                                perf_mode=DR)
                    st = spool.tile([128, 512], f32, name="st", tag="st")
                    nc.vector.tensor_scalar_mul(st, ps, 1.0 / (SY * SW))
                    nc.sync.dma_start(
                        out_d[tg * 128:(tg + 1) * 128, so], st)
                sched["d"] -= (512 + 2 * 1536) * PE_NS

            def drain(thresh=0.0):
                while fq and sched["d"] > thresh:
                    fq.pop(0)["fn"]()

            def ensure(key):
                for idx, ent in enumerate(fq):
                    if ent["key"] == key:
                        fq.pop(idx)["fn"]()
                        return

            def attn_head(i, p, hh):
                h = 2 * p + hh
                kt_hi = 4 * i + 3
                q_ap = qk_sb[hh * 64:(hh + 1) * 64, p, :]
                k_ap = qk_sb[hh * 64:(hh + 1) * 64, 4 + p, :]
                y_ps = ps_y.tile([128, 4, 66], f32, name="yps", tag="yps")
                pend = None   # (kt, e, r)

                def emit_y(kt, e, r):
                    for sub in range(max(0, r), 4):
                        nc.tensor.matmul(
                            y_ps[:, sub, 0:HD + 1],
                            lhsT=e[:, sub * 128:(sub + 1) * 128],
                            rhs=vt_sb[:, kt, h, 0:HD + 1],
                            start=(kt == 0 and sub == max(0, r)),
                            stop=(kt == kt_hi and sub == 3))
                    sched["d"] -= (4 - max(0, r)) * 65 * PE_NS

                for kt in range(kt_hi + 1):
                    r = kt - 4 * i
                    c0 = 128 * r if r > 0 else 0
                    s_ps = ps_s.tile([128, QT], f32, name="sps", tag="sps")
                    nc.tensor.matmul(
                        s_ps[:, c0:], lhsT=k_ap[:, kt * 128:(kt + 1) * 128],
                        rhs=q_ap[:, i * QT + c0:(i + 1) * QT],
                        start=True, stop=True)
                    sched["d"] -= (QT - c0) * PE_NS
                    e = epool.tile([128, QT], b16, name="e", tag="e")
                    nc.scalar.activation(e[:, c0:], s_ps[:, c0:], EXP, scale=0.125)
                    sched["d"] += (QT - c0) * ACT_NS + ACT_OVH
                    if r >= 0:
                        nc.vector.tensor_mul(e[:, c0:c0 + 128], e[:, c0:c0 + 128], tri_sb)
                    if pend is not None:
                        emit_y(*pend)
                    pend = (kt, e, r)
                    drain()
                emit_y(*pend)
                rc = rpool.tile([128, 4], f32, name="rc", tag="rc")
                nc.vector.reciprocal(rc, y_ps[:, :, HD])
                for sub in range(4):
                    tg = 4 * i + sub
                    nc.vector.tensor_scalar_mul(
                        y_tiles[tg][:, h, :], y_ps[:, sub, 0:HD], rc[:, sub:sub + 1])

            # ---- emission ----
            # queue all future proj groups as fillers (in dependency-safe order)
            for i in range(NST):
                for jt in (0, 4, 1, 5, 2, 6, 3, 7):
                    fq.append({"key": ("qk", i, jt),
                               "fn": (lambda jt=jt, i=i: qk_group(jt, i))})
                for tt in range(4):
                    fq.append({"key": ("v", i, tt),
                               "fn": (lambda tt=tt, i=i: v_group(tt, i))})

            for i in range(NST):
                for p in range(4):
                    ensure(("qk", i, p))
                    ensure(("qk", i, 4 + p))
                    if p == 0:
                        for tt in range(4):
                            ensure(("v", i, tt))
                    for hh in range(2):
                        attn_head(i, p, hh)
                        drain()
                # step's tp/P4 becomes deferred filler
                for tt in range(4):
                    fq.append({"key": ("p4", i, tt),
                               "fn": (lambda tg=4 * i + tt: tp_p4(tg))})
            while fq:
                fq.pop(0)["fn"]()

    nc.compile()
    return nc


def _split8(m):
    hi = m.astype(f8)
    lo = (m - hi.astype(np.float32)).astype(f8)
    return hi, lo


def _prep_inputs(x, w_attn, w_proj):
    # tri[kl, ql] = 1 if ql >= kl (keep), else 0 (causal-masked)
    tri = np.ascontiguousarray(np.triu(np.ones((128, 128), np.float32))).astype(bf16)
    ident = np.ascontiguousarray(np.eye(128, dtype=np.float32)).astype(bf16)
    in_maps = []
    for core in range(N_CORES):
        b, g = core // 2, core % 2
        heads = [8 * g + i for i in range(H_LOC)]
        q_rows = np.concatenate([w_attn[HD * h:HD * h + HD] for h in heads])
        k_rows = np.concatenate([w_attn[C + HD * h:C + HD * h + HD] for h in heads])
        v_rows = np.concatenate([w_attn[2 * C + HD * h:2 * C + HD * h + HD] for h in heads])
        wqk = np.ascontiguousarray(np.concatenate([q_rows, k_rows]).T)
        wv = np.ascontiguousarray(v_rows.T)
        wp = np.ascontiguousarray(
            np.concatenate([w_proj[:, HD * h:HD * h + HD] for h in heads], axis=1).T)
        xT = np.ascontiguousarray(x[b].T)
        xh, xl = _split8(xT * SX)
        wqkh, wqkl = _split8(wqk * SW)
        wvh, wvl = _split8(wv * SW)
        wph, wpl = _split8(wp * SW)
        in_maps.append({
            "xh": xh, "xl": xl, "wqkh": wqkh, "wqkl": wqkl,
            "wvh": wvh, "wvl": wvl, "wph": wph, "wpl": wpl,
            "tri": tri, "ident": ident})
    return in_maps


def kernel(x, w_attn, w_proj):
    from concourse.bass_utils import run_bass_kernel_spmd

    x = np.asarray(x, dtype=np.float32)
    w_attn = np.asarray(w_attn, dtype=np.float32)
    w_proj = np.asarray(w_proj, dtype=np.float32)

    if "nc" not in _CACHE:
        _CACHE["nc"] = _build()
    nc = _CACHE["nc"]

    in_maps = _prep_inputs(x, w_attn, w_proj)
    res = run_bass_kernel_spmd(nc, in_maps, core_ids=list(range(N_CORES)))
    outs = [res.results[c]["out"] for c in range(N_CORES)]
    y = np.stack([outs[2 * b] + outs[2 * b + 1] for b in range(B)])
    return y.astype(np.float32)


# revision 13
# speedup vs baseline: 1.6022x; 1.0498x over previous
"""Causal self-attention (B=4, T=2048, C=1024, H=16) on 8 trn2 NeuronCores.

Sharding: core = 2*b + g  (b = batch 0..3, g = head-group 0..1).
Each core handles 1 batch and 8 heads and returns a partial projection
output [T, C]; the host sums the two head-group partials per batch.

v3: flash-style q-superstep ordering + transposed-Y attention + fp8
DoubleRow 3-term-compensated projections (QK, V, P4) + deficit-scheduled
PE filler interleave so projection/P4 matmuls hide the Act-engine-bound
exp phase.

Per-superstep i (512 q):
  P1  project Q,K (feature-major [128j, t]) and V (token-major [t, j]) for
      this t-range via fp8 DR matmuls:  x_hi@W_hi + x_lo@W_hi + x_hi@W_lo,
      each term 4 DoubleRow matmuls (256-deep contraction, 0.5 cyc/row).
  P3  per (head, kt<=4i+3): S_T = K.T Q [128k, 512q] (bf16), E = exp(S/8)
      on Act, tri-mask diag blocks on DVE; per 128-q subtile accumulate
      y[q, 64d | rowsum] += E_blk.T @ [V|1] (transposed-Y: 65 rows/block).
      S runs one kt ahead of Y; fillers (future proj / deferred P4) are
      drained whenever the Act-minus-PE deficit estimate goes positive.
      Normalize: one DVE reciprocal [128,4] per (head, step) + scalar-mul.
  P5  (deferred, as filler) transpose y [t,j]->[j,t] on PE, split yT into
      fp8 hi/lo on DVE, P4 out = yT.T @ wp via fp8 DR matmuls, stream out.
"""

import numpy as np
import ml_dtypes

B, T, C = 4, 2048, 1024
H_LOC = 8          # heads per core
HD = 64            # head dim
N_CORES = 8
QT = 512           # q super-step width
NST = T // QT      # 4
NCT = C // 128     # 8 contraction chunks for qkv

bf16 = ml_dtypes.bfloat16
f8 = ml_dtypes.float8_e4m3

# fp8 hi/lo splits are pre-scaled so the lo residuals stay in e4m3's normal
# range (unscaled weight residuals ~1e-3 fall below the 2^-9 min subnormal
# and flush to zero, killing the compensation).  Products carry SX*SW=512;
# PSUM evacuations divide it back out.
SX = 8.0           # x pre-scale
SW = 64.0          # weight pre-scale
SY = 8.0           # on-device y pre-scale for the P4 hi/lo split

PE_NS = 0.417      # ns per output row at full pstate
ACT_NS = 0.833
ACT_OVH = 185.0

_CACHE = {}


def _build():
    import concourse.bacc as bacc
    import concourse.tile as tile
    import concourse.mybir as mybir
    from contextlib import ExitStack

    f32 = mybir.dt.float32
    b16 = mybir.dt.bfloat16
    fp8 = mybir.dt.float8e4
    EXP = mybir.ActivationFunctionType.Exp
    DR = mybir.MatmulPerfMode.DoubleRow

    nc = bacc.Bacc("TRN2", target_bir_lowering=False, debug=False)

    xh_d = nc.dram_tensor("xh", [C, T], fp8, kind="ExternalInput").ap()
    xl_d = nc.dram_tensor("xl", [C, T], fp8, kind="ExternalInput").ap()
    wqkh_d = nc.dram_tensor("wqkh", [C, 1024], fp8, kind="ExternalInput").ap()
    wqkl_d = nc.dram_tensor("wqkl", [C, 1024], fp8, kind="ExternalInput").ap()
    wvh_d = nc.dram_tensor("wvh", [C, 512], fp8, kind="ExternalInput").ap()
    wvl_d = nc.dram_tensor("wvl", [C, 512], fp8, kind="ExternalInput").ap()
    wph_d = nc.dram_tensor("wph", [512, C], fp8, kind="ExternalInput").ap()
    wpl_d = nc.dram_tensor("wpl", [512, C], fp8, kind="ExternalInput").ap()
    tri_d = nc.dram_tensor("tri", [128, 128], b16, kind="ExternalInput").ap()
    ident_d = nc.dram_tensor("ident", [128, 128], b16, kind="ExternalInput").ap()
    out_d = nc.dram_tensor("out", [T, C], f32, kind="ExternalOutput").ap()

    with tile.TileContext(nc) as tc:
        with ExitStack() as ctx:
            pers = ctx.enter_context(tc.tile_pool(name="pers", bufs=1))
            xh_sb = pers.tile([128, NCT, T], fp8)
            xl_sb = pers.tile([128, NCT, T], fp8)
            wqkh_sb = pers.tile([128, NCT, 1024], fp8)
            wqkl_sb = pers.tile([128, NCT, 1024], fp8)
            wvh_sb = pers.tile([128, NCT, 512], fp8)
            wvl_sb = pers.tile([128, NCT, 512], fp8)
            wph_sb = pers.tile([128, 4, C], fp8)
            wpl_sb = pers.tile([128, 4, C], fp8)
            tri_sb = pers.tile([128, 128], b16)
            id_sb = pers.tile([128, 128], b16)
            qk_sb = pers.tile([128, 8, T], b16)   # jt 0..3 Q pairs, 4..7 K pairs
            vt_sb = pers.tile([128, T // 128, H_LOC, HD + 2], b16)  # ones col @64
            y_tiles = [pers.tile([128, H_LOC, HD], b16, name=f"y{t}")
                       for t in range(T // 128)]

            nc.gpsimd.memset(vt_sb[:, :, :, HD], 1.0)

            xh_r = xh_d.rearrange("(a p) t -> p a t", p=128)
            xl_r = xl_d.rearrange("(a p) t -> p a t", p=128)
            wqkh_r = wqkh_d.rearrange("(a p) j -> p a j", p=128)
            wqkl_r = wqkl_d.rearrange("(a p) j -> p a j", p=128)
            wvh_r = wvh_d.rearrange("(a p) j -> p a j", p=128)
            wvl_r = wvl_d.rearrange("(a p) j -> p a j", p=128)
            wph_r = wph_d.rearrange("(c p) j -> p c j", p=128)
            wpl_r = wpl_d.rearrange("(c p) j -> p c j", p=128)

            # step-0 x and pair-0 wqk columns first so attention starts early
            nc.sync.dma_start(xh_sb[:, :, 0:QT], xh_r[:, :, 0:QT])
            for jt in (0, 4):
                c = slice(jt * 128, (jt + 1) * 128)
                nc.sync.dma_start(wqkh_sb[:, :, c], wqkh_r[:, :, c])
                nc.sync.dma_start(wqkl_sb[:, :, c], wqkl_r[:, :, c])
            nc.sync.dma_start(xl_sb[:, :, 0:QT], xl_r[:, :, 0:QT])
            nc.sync.dma_start(tri_sb, tri_d)
            nc.sync.dma_start(id_sb, ident_d)
            for jt in (1, 5, 2, 6, 3, 7):
                c = slice(jt * 128, (jt + 1) * 128)
                nc.sync.dma_start(wqkh_sb[:, :, c], wqkh_r[:, :, c])
                nc.sync.dma_start(wqkl_sb[:, :, c], wqkl_r[:, :, c])
                if jt == 5:
                    nc.sync.dma_start(wvh_sb, wvh_r)
                    nc.sync.dma_start(wvl_sb, wvl_r)
            for i in range(1, NST):
                s = slice(i * QT, (i + 1) * QT)
                nc.sync.dma_start(xh_sb[:, :, s], xh_r[:, :, s])
                nc.sync.dma_start(xl_sb[:, :, s], xl_r[:, :, s])
            nc.sync.dma_start(wph_sb, wph_r)
            nc.sync.dma_start(wpl_sb, wpl_r)

            epool = ctx.enter_context(tc.tile_pool(name="epool", bufs=4))
            ytp_pool = ctx.enter_context(tc.tile_pool(name="ytp", bufs=2))
            rpool = ctx.enter_context(tc.tile_pool(name="rpool", bufs=4))
            spool = ctx.enter_context(tc.tile_pool(name="spool", bufs=4))
            ps_big = ctx.enter_context(tc.tile_pool(name="psbig", bufs=2, space="PSUM"))
            ps_s = ctx.enter_context(tc.tile_pool(name="pss", bufs=3, space="PSUM"))
            ps_y = ctx.enter_context(tc.tile_pool(name="psy", bufs=2, space="PSUM"))
            ps_t = ctx.enter_context(tc.tile_pool(name="pst", bufs=1, space="PSUM"))

            # ---- deficit-based filler scheduler (generator-chunked) ----
            # Fillers are generators yielding every ~3 matmuls (~300-400ns of
            # PE work) so drain() can match the ~250ns/block Act-PE imbalance
            # without starving either engine.
            sched = {"d": 0.0}
            fq = []          # list of dicts: key, fn (generator factory)
            cur = {"g": None, "key": None}

            def dr3(ps, lhs_h, lhs_l, rhs_h, rhs_l):
                """3-term fp8 DoubleRow GEMM over 1024-deep contraction;
                yields every 3 matmuls."""
                terms = ((lhs_h, rhs_h), (lhs_l, rhs_h), (lhs_h, rhs_l))
                n = 0
                for ti, (lt, rt) in enumerate(terms):
                    for a4 in range(4):
                        sl = slice(2 * a4, 2 * a4 + 2)
                        nc.tensor.matmul(
                            ps, lhsT=lt[:, sl, :], rhs=rt[:, sl, :],
                            start=(ti == 0 and a4 == 0),
                            stop=(ti == 2 and a4 == 3),
                            perf_mode=DR)
                        sched["d"] -= 256 * PE_NS
                        n += 1
                        if n % 3 == 0:
                            yield

            def qk_group(jt, i):
                t0 = i * QT
                c = slice(jt * 128, (jt + 1) * 128)
                s = slice(t0, t0 + QT)
                ps = ps_big.tile([128, QT], f32, name="pqk", tag="big")
                yield from dr3(ps, wqkh_sb[:, :, c], wqkl_sb[:, :, c],
                               xh_sb[:, :, s], xl_sb[:, :, s])
                nc.vector.tensor_scalar_mul(qk_sb[:, jt, s], ps, 1.0 / (SX * SW))

            def v_group(tt, i):
                tg = 4 * i + tt
                s = slice(tg * 128, (tg + 1) * 128)
                ps = ps_big.tile([128, H_LOC * HD], f32, name="pv", tag="big")
                yield from dr3(ps, xh_sb[:, :, s], xl_sb[:, :, s], wvh_sb, wvl_sb)
                nc.vector.tensor_scalar_mul(
                    vt_sb[:, tg, :, 0:HD], ps.rearrange("p (h d) -> p h d", d=HD),
                    1.0 / (SX * SW))

            def tp_p4(tg):
                yflat = y_tiles[tg].rearrange("p h d -> p (h d)")
                tp = ps_t.tile([128, 4, 128], b16, name="tp", tag="tp")
                for jc in range(4):
                    nc.tensor.transpose(
                        tp[:, jc, :], yflat[:, jc * 128:(jc + 1) * 128], id_sb)
                sched["d"] -= 512 * PE_NS
                yth = ytp_pool.tile([128, 4, 128], fp8, name="yth", tag="yth")
                ytl = ytp_pool.tile([128, 4, 128], fp8, name="ytl", tag="ytl")
                nc.vector.tensor_scalar_mul(yth, tp, SY)
                nc.vector.scalar_tensor_tensor(
                    ytl, tp, SY, yth, mybir.AluOpType.mult, mybir.AluOpType.subtract)
                yield
                for ot in range(2):
                    so = slice(ot * 512, (ot + 1) * 512)
                    ps = ps_big.tile([128, 512], f32, name="po", tag="big")
                    terms = ((yth, wph_sb), (ytl, wph_sb), (yth, wpl_sb))
                    n = 0
                    for ti, (lt, rt) in enumerate(terms):
                        for c4 in range(2):
                            sl = slice(2 * c4, 2 * c4 + 2)
                            nc.tensor.matmul(
                                ps, lhsT=lt[:, sl, :], rhs=rt[:, sl, so],
                                start=(ti == 0 and c4 == 0),
                                stop=(ti == 2 and c4 == 1),
                                perf_mode=DR)
                            sched["d"] -= 256 * PE_NS
                            n += 1
                            if n % 3 == 0:
                                yield
                    st = spool.tile([128, 512], f32, name="st", tag="st")
                    nc.vector.tensor_scalar_mul(st, ps, 1.0 / (SY * SW))
                    nc.sync.dma_start(
                        out_d[tg * 128:(tg + 1) * 128, so], st)

            def _advance():
                """Run one chunk of the current/next filler. False if empty."""
                if cur["g"] is None:
                    if not fq:
                        return False
                    ent = fq.pop(0)
                    cur["g"] = ent["fn"]()
                    cur["key"] = ent["key"]
                try:
                    next(cur["g"])
                except StopIteration:
                    cur["g"] = None
                    cur["key"] = None
                return True

            def drain(thresh=0.0):
                while sched["d"] > thresh:
                    if not _advance():
                        return

            def _exhaust(gen):
                for _ in gen:
                    pass

            def ensure(key):
                if cur["key"] == key:
                    _exhaust(cur["g"])
                    cur["g"] = None
                    cur["key"] = None
                    return
                for idx, ent in enumerate(fq):
                    if ent["key"] == key:
                        _exhaust(fq.pop(idx)["fn"]())
                        return

            def attn_head(i, p, hh):
                h = 2 * p + hh
                kt_hi = 4 * i + 3
                q_ap = qk_sb[hh * 64:(hh + 1) * 64, p, :]
                k_ap = qk_sb[hh * 64:(hh + 1) * 64, 4 + p, :]
                y_ps = ps_y.tile([128, 4, 66], f32, name="yps", tag="yps")
                pend = None   # (kt, e, r)

                def emit_y(kt, e, r):
                    for sub in range(max(0, r), 4):
                        nc.tensor.matmul(
                            y_ps[:, sub, 0:HD + 1],
                            lhsT=e[:, sub * 128:(sub + 1) * 128],
                            rhs=vt_sb[:, kt, h, 0:HD + 1],
                            start=(kt == 0 and sub == max(0, r)),
                            stop=(kt == kt_hi and sub == 3))
                    sched["d"] -= (4 - max(0, r)) * 65 * PE_NS

                for kt in range(kt_hi + 1):
                    r = kt - 4 * i
                    c0 = 128 * r if r > 0 else 0
                    s_ps = ps_s.tile([128, QT], f32, name="sps", tag="sps")
                    nc.tensor.matmul(
                        s_ps[:, c0:], lhsT=k_ap[:, kt * 128:(kt + 1) * 128],
                        rhs=q_ap[:, i * QT + c0:(i + 1) * QT],
                        start=True, stop=True)
                    sched["d"] -= (QT - c0) * PE_NS
                    e = epool.tile([128, QT], b16, name="e", tag="e")
                    nc.scalar.activation(e[:, c0:], s_ps[:, c0:], EXP, scale=0.125)
                    sched["d"] += (QT - c0) * ACT_NS + ACT_OVH
                    if r >= 0:
                        nc.vector.tensor_mul(e[:, c0:c0 + 128], e[:, c0:c0 + 128], tri_sb)
                    if pend is not None:
                        emit_y(*pend)
                    pend = (kt, e, r)
                    drain()
                emit_y(*pend)
                rc = rpool.tile([128, 4], f32, name="rc", tag="rc")
                nc.vector.reciprocal(rc, y_ps[:, :, HD])
                for sub in range(4):
                    tg = 4 * i + sub
                    nc.vector.tensor_scalar_mul(
                        y_tiles[tg][:, h, :], y_ps[:, sub, 0:HD], rc[:, sub:sub + 1])

            # ---- emission ----
            # queue all future proj groups as fillers (in dependency-safe order)
            for i in range(NST):
                for jt in (0, 4, 1, 5, 2, 6, 3, 7):
                    fq.append({"key": ("qk", i, jt),
                               "fn": (lambda jt=jt, i=i: qk_group(jt, i))})
                for tt in range(4):
                    fq.append({"key": ("v", i, tt),
                               "fn": (lambda tt=tt, i=i: v_group(tt, i))})

            for i in range(NST):
                for p in range(4):
                    ensure(("qk", i, p))
                    ensure(("qk", i, 4 + p))
                    if p == 0:
                        for tt in range(4):
                            ensure(("v", i, tt))
                    for hh in range(2):
                        attn_head(i, p, hh)
                        drain()
                # step's tp/P4 becomes deferred filler
                for tt in range(4):
                    fq.append({"key": ("p4", i, tt),
                               "fn": (lambda tg=4 * i + tt: tp_p4(tg))})
            if cur["g"] is not None:
                _exhaust(cur["g"])
            while fq:
                _exhaust(fq.pop(0)["fn"]())

    nc.compile()
    return nc


def _split8(m):
    hi = m.astype(f8)
    lo = (m - hi.astype(np.float32)).astype(f8)
    return hi, lo


def _prep_inputs(x, w_attn, w_proj):
    # tri[kl, ql] = 1 if ql >= kl (keep), else 0 (causal-masked)
    tri = np.ascontiguousarray(np.triu(np.ones((128, 128), np.float32))).astype(bf16)
    ident = np.ascontiguousarray(np.eye(128, dtype=np.float32)).astype(bf16)
    in_maps = []
    for core in range(N_CORES):
        b, g = core // 2, core % 2
        heads = [8 * g + i for i in range(H_LOC)]
        q_rows = np.concatenate([w_attn[HD * h:HD * h + HD] for h in heads])
        k_rows = np.concatenate([w_attn[C + HD * h:C + HD * h + HD] for h in heads])
        v_rows = np.concatenate([w_attn[2 * C + HD * h:2 * C + HD * h + HD] for h in heads])
        wqk = np.ascontiguousarray(np.concatenate([q_rows, k_rows]).T)
        wv = np.ascontiguousarray(v_rows.T)
        wp = np.ascontiguousarray(
            np.concatenate([w_proj[:, HD * h:HD * h + HD] for h in heads], axis=1).T)
        xT = np.ascontiguousarray(x[b].T)
        xh, xl = _split8(xT * SX)
        wqkh, wqkl = _split8(wqk * SW)
        wvh, wvl = _split8(wv * SW)
        wph, wpl = _split8(wp * SW)
        in_maps.append({
            "xh": xh, "xl": xl, "wqkh": wqkh, "wqkl": wqkl,
            "wvh": wvh, "wvl": wvl, "wph": wph, "wpl": wpl,
            "tri": tri, "ident": ident})
    return in_maps


def kernel(x, w_attn, w_proj):
    from concourse.bass_utils import run_bass_kernel_spmd

    x = np.asarray(x, dtype=np.float32)
    w_attn = np.asarray(w_attn, dtype=np.float32)
    w_proj = np.asarray(w_proj, dtype=np.float32)

    if "nc" not in _CACHE:
        _CACHE["nc"] = _build()
    nc = _CACHE["nc"]

    in_maps = _prep_inputs(x, w_attn, w_proj)
    res = run_bass_kernel_spmd(nc, in_maps, core_ids=list(range(N_CORES)))
    outs = [res.results[c]["out"] for c in range(N_CORES)]
    y = np.stack([outs[2 * b] + outs[2 * b + 1] for b in range(B)])
    return y.astype(np.float32)
